# revision 4
# baseline (speedup 1.0000x reference)
"""InnerAttention kernel for 8 Trainium2 NeuronCores.

Computes, per batch b:
    e = x[b] @ y[b].T          [M, N]
    p = softmax(e, axis=-1)    (over n)
    out[b] = p.T @ x[b]        [N, D]

Sharding: data-parallel over batch (B=8 -> one batch per core). Full inputs
in, full output out. The host-side wrapper pre-casts to fp16 and pre-builds
device-friendly layouts with numpy so every DMA is 128 partitions x >=1KB
contiguous per partition:
  xTs [P, 16, 8, P] fp16  (xTs[p,mi,k,c] = x[128mi+c, 128k+p]; one 2KB-per-
                           partition DMA per m-tile -> mm1 stationary chunks)
  x16 [M, D] fp16         (x natural, mm2 moving after 1/s scaling)
  yT  [D, N] fp16         (y transposed, mm1 moving; loaded as 32 column-
                           slice tiles [128, 512] gated individually)

Per-core (M=N=2048, D=1024, P=128):
  mm1: per m-tile, e[128, 2048] in PSUM (4 banks of [128,512] fp32), fp16
       single pass; m-tiles 0/1 interleave into the yT slice-DMA window,
       each k-step gated only on its own [128,512] yT slice arriving.
  softmax: DVE row-max over PSUM, ACT exp (bias=-max) -> p fp16 in SBUF,
       accum_out row-sum; 1/sum folded into xs = x16 * (1/s) (fp16).
  mm2: per (n-chunk 128, d-half 512) out tile: accumulate all 16
       p.T @ xs contributions in one PSUM bank; stage the two 256-col
       halves in parallel on DVE and ACT, DMA each half as it lands
       (ACT-half DMAs issue on the Activation HWDGE queue).
  Clock warmup: PE ramps 1.2->2.4GHz only after ~3us of continuous work;
       dummy matmuls burn the ramp during the initial DMA fill, read back
       straight from PSUM late in the SP DMA queue.
"""

import numpy as np

import concourse.bacc as bacc
import concourse.mybir as mybir
import concourse.tile as tile
from concourse import bass_utils

B, M, N, D = 8, 2048, 2048, 1024
P = 128
NSLICE = 512
N_MTILES = M // P     # 16
N_DCHUNK = D // P     # 8
N_NSL = N // NSLICE   # 4
N_NCHUNK = N // P     # 16
N_DHALF = D // NSLICE  # 2
NWARM = 16

F32 = mybir.dt.float32
FP16 = mybir.dt.float16
AX = mybir.AxisListType.X
EXP = mybir.ActivationFunctionType.Exp


def _build_nc():
    nc = bacc.Bacc("TRN2", target_bir_lowering=False, debug=False)
    xTs_d = nc.dram_tensor("xTs", [P, N_MTILES, N_DCHUNK, P], FP16,
                           kind="ExternalInput").ap()
    x16_d = nc.dram_tensor("x16", [M, D], FP16, kind="ExternalInput").ap()
    yT_d = nc.dram_tensor("yT", [D, N], FP16, kind="ExternalInput").ap()
    out_d = nc.dram_tensor("out", [N, D], F32, kind="ExternalOutput").ap()
    warm_d = nc.dram_tensor("warm_o", [P, 4], F32, kind="ExternalOutput").ap()

    with tile.TileContext(nc) as tc:
        with (
            tc.tile_pool(name="yTp", bufs=1) as yTp,
            tc.tile_pool(name="pP", bufs=1) as pPp,
            tc.tile_pool(name="xsP", bufs=1) as xsPp,
            tc.tile_pool(name="work", bufs=2) as work,
            tc.tile_pool(name="stats", bufs=3) as stats,
            tc.tile_pool(name="eps", bufs=8, space="PSUM") as epsp,
        ):
            # yT column-slice tiles: yts[k][ns][p, j] = y[512ns+j, 128k+p]
            yts = [[yTp.tile([P, NSLICE], FP16, tag=f"yt{k}_{ns}",
                             name=f"yt{k}_{ns}")
                    for ns in range(N_NSL)] for k in range(N_DCHUNK)]
            pT = [pPp.tile([P, N], FP16, tag=f"prob{mi}", name=f"prob{mi}")
                  for mi in range(N_MTILES)]
            xs = [xsPp.tile([P, D], FP16, tag=f"xsc{mi}", name=f"xsc{mi}")
                  for mi in range(N_MTILES)]

            xT_of = {}
            x16_of = {}

            def emit_xT(mi):
                if mi >= N_MTILES:
                    return
                xT = work.tile([P, N_DCHUNK, P], FP16, tag="xT", bufs=4)
                nc.sync.dma_start(xT[:], xTs_d[:, mi, :, :])
                xT_of[mi] = xT

            def emit_x16(mi):
                if mi >= N_MTILES:
                    return
                x16 = work.tile([P, D], FP16, tag="x16", bufs=4)
                nc.sync.dma_start(x16[:], x16_d[mi * P:(mi + 1) * P, :])
                x16_of[mi] = x16

            def emit_yslice(k, ns):
                nc.sync.dma_start(
                    yts[k][ns][:],
                    yT_d[k * P:(k + 1) * P, ns * NSLICE:(ns + 1) * NSLICE])

            def emit_mm1_group(mi, ns):
                xT = xT_of[mi]
                ep = epsp.tile([P, NSLICE], F32, tag="e", name="eps")
                for k in range(N_DCHUNK):
                    nc.tensor.matmul(
                        ep[:], xT[:, k, :], yts[k][ns][:],
                        start=(k == 0), stop=(k == N_DCHUNK - 1),
                    )
                return ep

            def emit_softmax(mi, eps):
                rmax4 = stats.tile([P, N_NSL], F32, tag="rmax4")
                for ns in range(N_NSL):
                    nc.vector.reduce_max(rmax4[:, ns:ns + 1], eps[ns][:], axis=AX)
                negmax = stats.tile([P, 1], F32, tag="negmax")
                rmax = stats.tile([P, 1], F32, tag="rmax")
                nc.vector.reduce_max(rmax[:], rmax4[:], axis=AX)
                nc.vector.tensor_scalar_mul(negmax[:], rmax[:], -1.0)

                s4 = stats.tile([P, N_NSL], F32, tag="s4")
                for ns in range(N_NSL):
                    c0 = ns * NSLICE
                    nc.scalar.activation(
                        pT[mi][:, c0:c0 + NSLICE], eps[ns][:], EXP,
                        bias=negmax[:], accum_out=s4[:, ns:ns + 1],
                    )
                ssum = stats.tile([P, 1], F32, tag="ssum")
                nc.vector.reduce_sum(ssum[:], s4[:], axis=AX)
                rinv = stats.tile([P, 1], F32, tag="rinv")
                nc.vector.reciprocal(rinv[:], ssum[:])
                nc.vector.tensor_scalar_mul(xs[mi][:], x16_of.pop(mi)[:], rinv[:])
                del xT_of[mi]

            # ---- clock warmup: the PE ramps 1.2->2.4GHz only after ~3us
            # of continuous work; burn the ramp on dummy matmuls during
            # the otherwise-idle initial DMA fill. ----
            wsrc = work.tile([P, P], FP16, tag="wsrc", bufs=1)
            nc.vector.memset(wsrc[:], 0.0)
            wps = epsp.tile([P, NSLICE], F32, tag="e", name="warmps")
            for i in range(NWARM):
                nc.tensor.matmul(wps[:, 0:P], wsrc[:], wsrc[:],
                                 start=(i == 0), stop=(i == NWARM - 1))

            # ---- opening DMA stream (all on the SP HWDGE queue, in this
            # order): xT0, ns0 slices (xT1 interleaved), ns1, x16 0/1,
            # ns2, ns3, xT/x16 2/3, warm readback. ----
            emit_xT(0)
            for k in range(N_DCHUNK):
                emit_yslice(k, 0)
                if k == 3:
                    emit_xT(1)
            for k in range(N_DCHUNK):
                emit_yslice(k, 1)
            emit_x16(0)
            emit_x16(1)
            for k in range(N_DCHUNK):
                emit_yslice(k, 2)
            for k in range(N_DCHUNK):
                emit_yslice(k, 3)
            emit_xT(2)
            emit_x16(2)
            emit_xT(3)
            emit_x16(3)
            # warm readback staged via the idle GPSIMD engine; its DMA sits
            # in the SP queue after all opening DMAs so its sem-wait never
            # delays them.
            wstage = stats.tile([P, 4], F32, tag="wstage")
            nc.scalar.copy(wstage[:], wps[:, 0:4])
            nc.sync.dma_start(warm_d, wstage[:])

            # ---- opening: tiles 0/1 interleave into the slice-DMA window,
            # each k-step gated only on its own [128,512] slice. ----
            eps0 = [emit_mm1_group(0, ns) for ns in range(N_NSL)]
            eps1 = [emit_mm1_group(1, ns) for ns in range(N_NSL)]
            emit_softmax(0, eps0)
            emit_softmax(1, eps1)

            # ---- steady state ----
            for mi in range(2, N_MTILES):
                eps = []
                for ns in range(N_NSL):
                    eps.append(emit_mm1_group(mi, ns))
                    if ns == 0:
                        emit_xT(mi + 2)
                        emit_x16(mi + 2)
                emit_softmax(mi, eps)

            # ---- mm2: per group, stage the two 256-col halves on DVE and
            # ACT in parallel, DMA each half as soon as it lands. ----
            HS = NSLICE // 2
            for nch in range(N_NCHUNK):
                for dh in range(N_DHALF):
                    ops = epsp.tile([P, NSLICE], F32, tag="e", name="ops")
                    for mi in range(N_MTILES):
                        nc.tensor.matmul(
                            ops[:],
                            pT[mi][:, nch * P:(nch + 1) * P],
                            xs[mi][:, dh * NSLICE:(dh + 1) * NSLICE],
                            start=(mi == 0), stop=(mi == N_MTILES - 1),
                        )
                    rows = slice(nch * P, (nch + 1) * P)
                    c0 = dh * NSLICE
                    hA = work.tile([P, HS], F32, tag="ostgA", bufs=4)
                    hB = work.tile([P, HS], F32, tag="ostgB", bufs=4)
                    nc.vector.tensor_copy(hA[:], ops[:, 0:HS])
                    nc.scalar.copy(hB[:], ops[:, HS:NSLICE])
                    nc.sync.dma_start(out_d[rows, c0:c0 + HS], hA[:])
                    nc.scalar.dma_start(out_d[rows, c0 + HS:c0 + NSLICE], hB[:])

    nc.compile()
    return nc


_NC_CACHE = {}


def _get_nc():
    if "nc" not in _NC_CACHE:
        _NC_CACHE["nc"] = _build_nc()
    return _NC_CACHE["nc"]


def _host_inputs(x_b: np.ndarray, y_b: np.ndarray) -> dict:
    x16 = x_b.astype(np.float16)
    xTs = np.ascontiguousarray(
        x16.reshape(N_MTILES, P, N_DCHUNK, P).transpose(3, 0, 2, 1))
    return {
        "xTs": xTs,
        "x16": np.ascontiguousarray(x16),
        "yT": np.ascontiguousarray(y_b.astype(np.float16).T),
    }


def kernel(x: np.ndarray, y: np.ndarray) -> np.ndarray:
    assert x.shape == (B, M, D) and y.shape == (B, N, D)
    nc = _get_nc()
    in_maps = [_host_inputs(x[b], y[b]) for b in range(B)]
    res = bass_utils.run_bass_kernel_spmd(nc, in_maps, core_ids=list(range(B)))
    return np.stack([res.results[b]["out"] for b in range(B)], axis=0)


# revision 5
# speedup vs baseline: 1.0129x; 1.0129x over previous
"""InnerAttention kernel for 8 Trainium2 NeuronCores.

Computes, per batch b:
    e = x[b] @ y[b].T          [M, N]
    p = softmax(e, axis=-1)    (over n)
    out[b] = p.T @ x[b]        [N, D]

Sharding: data-parallel over batch (B=8 -> one batch per core). Full inputs
in, full output out. The host-side wrapper pre-casts to fp16 and pre-builds
device-friendly layouts with numpy so every DMA is 128 partitions x >=2KB
contiguous per partition:
  xTs [P, 16, 8, P] fp16  (xTs[p,mi,k,c] = x[128mi+c, 128k+p]; mm1
                           stationary chunks, loaded 1 m-tile per DMA for
                           tiles 0-3 and 4 m-tiles per DMA for 4-15)
  x16 [M, D] fp16         (x natural; mm2 moving after 1/s scaling; loaded
                           4 m-tiles per DMA)
  yT  [D, N] fp16         (y transposed, mm1 moving; loaded as 4 DMAs of
                           [128, 2, 2048] = 2 d-chunks each)

HWDGE descriptor generation costs ~0.6us per dma_start and is serialized,
so the input stream is exactly 16 DMA instructions.

Per-core (M=N=2048, D=1024, P=128):
  mm1 opening: 8 PSUM accumulation chains (m-tiles 0/1 x 4 n-slices)
       interleaved k-step by k-step behind the 4 yT DMAs, so the PE
       saturates as soon as data drips in (PSUM accumulate is per-element
       has_written state, so interleaved chains to different banks are
       fine). Dummy matmuls before that burn the 1.2->2.4GHz clock ramp.
  softmax: DVE row-max over PSUM, ACT exp (bias=-max) -> p fp16 in SBUF,
       accum_out row-sum; 1/sum folded into xs = x16 * (1/s) (fp16).
  mm2: per (n-chunk 128, d-half 512) out tile: accumulate all 16
       p.T @ xs contributions in one PSUM bank; DVE and ACT copy one
       256-col half each in parallel into one staging tile, single DMA out.
"""

import numpy as np

import concourse.bacc as bacc
import concourse.mybir as mybir
import concourse.tile as tile
from concourse import bass_utils

B, M, N, D = 8, 2048, 2048, 1024
P = 128
NSLICE = 512
N_MTILES = M // P     # 16
N_DCHUNK = D // P     # 8
N_NSL = N // NSLICE   # 4
N_NCHUNK = N // P     # 16
N_DHALF = D // NSLICE  # 2
NWARM = 56
YKB = 2               # d-chunks per yT DMA
N_YB = N_DCHUNK // YKB  # 4

F32 = mybir.dt.float32
FP16 = mybir.dt.float16
AX = mybir.AxisListType.X
EXP = mybir.ActivationFunctionType.Exp


def _build_nc():
    nc = bacc.Bacc("TRN2", target_bir_lowering=False, debug=False)
    xTs_d = nc.dram_tensor("xTs", [P, N_MTILES, N_DCHUNK, P], FP16,
                           kind="ExternalInput").ap()
    x16_d = nc.dram_tensor("x16", [M, D], FP16, kind="ExternalInput").ap()
    yT_d = nc.dram_tensor("yT", [D, N], FP16, kind="ExternalInput").ap()
    out_d = nc.dram_tensor("out", [N, D], F32, kind="ExternalOutput").ap()
    warm_d = nc.dram_tensor("warm_o", [P, 4], F32, kind="ExternalOutput").ap()

    with tile.TileContext(nc) as tc:
        with (
            tc.tile_pool(name="yTp", bufs=1) as yTp,
            tc.tile_pool(name="pP", bufs=1) as pPp,
            tc.tile_pool(name="xsP", bufs=1) as xsPp,
            tc.tile_pool(name="xg", bufs=2) as xgp,
            tc.tile_pool(name="work", bufs=2) as work,
            tc.tile_pool(name="stats", bufs=3) as stats,
            tc.tile_pool(name="eps", bufs=8, space="PSUM") as epsp,
        ):
            # yT big tiles: yb[j][p, kk, n] = y[n, 128*(2j+kk)+p]
            yb = [yTp.tile([P, YKB, N], FP16, tag=f"yb{j}", name=f"yb{j}")
                  for j in range(N_YB)]
            pT = [pPp.tile([P, N], FP16, tag=f"prob{mi}", name=f"prob{mi}")
                  for mi in range(N_MTILES)]
            xs = [xsPp.tile([P, D], FP16, tag=f"xsc{mi}", name=f"xsc{mi}")
                  for mi in range(N_MTILES)]

            xT0_3 = {}
            xTg = {}
            x16g = {}
            yT_src = yT_d.rearrange("(a b p) n -> a p b n", b=YKB, p=P)
            x16_src = x16_d.rearrange("(g t p) d -> g p t d", t=4, p=P)

            def get_xT(mi, k):
                if mi < 4:
                    return xT0_3[mi][:, k, :]
                return xTg[mi // 4][:, mi % 4, k, :]

            def get_x16(mi):
                return x16g[mi // 4][:, mi % 4, :]

            def emit_mm1_group(mi, ns):
                ep = epsp.tile([P, NSLICE], F32, tag="e", name="eps")
                for k in range(N_DCHUNK):
                    nc.tensor.matmul(
                        ep[:], get_xT(mi, k),
                        yb[k // YKB][:, k % YKB, ns * NSLICE:(ns + 1) * NSLICE],
                        start=(k == 0), stop=(k == N_DCHUNK - 1),
                    )
                return ep

            def emit_softmax(mi, eps):
                rmax4 = stats.tile([P, N_NSL], F32, tag="rmax4")
                for ns in range(N_NSL):
                    nc.vector.reduce_max(rmax4[:, ns:ns + 1], eps[ns][:], axis=AX)
                negmax = stats.tile([P, 1], F32, tag="negmax")
                rmax = stats.tile([P, 1], F32, tag="rmax")
                nc.vector.reduce_max(rmax[:], rmax4[:], axis=AX)
                nc.vector.tensor_scalar_mul(negmax[:], rmax[:], -1.0)

                s4 = stats.tile([P, N_NSL], F32, tag="s4")
                for ns in range(N_NSL):
                    c0 = ns * NSLICE
                    nc.scalar.activation(
                        pT[mi][:, c0:c0 + NSLICE], eps[ns][:], EXP,
                        bias=negmax[:], accum_out=s4[:, ns:ns + 1],
                    )
                ssum = stats.tile([P, 1], F32, tag="ssum")
                nc.vector.reduce_sum(ssum[:], s4[:], axis=AX)
                rinv = stats.tile([P, 1], F32, tag="rinv")
                nc.vector.reciprocal(rinv[:], ssum[:])
                nc.vector.tensor_scalar_mul(xs[mi][:], get_x16(mi), rinv[:])

            # ---- clock warmup: the PE ramps 1.2->2.4GHz only after ~3us
            # of continuous work; burn the ramp on dummy matmuls during
            # the initial DMA fill. ----
            wsrc = work.tile([P, P], FP16, tag="wsrc", bufs=1)
            nc.vector.memset(wsrc[:], 0.0)
            wps = epsp.tile([P, NSLICE], F32, tag="e", name="warmps")
            for i in range(NWARM):
                nc.tensor.matmul(wps[:, 0:P], wsrc[:], wsrc[:],
                                 start=(i == 0), stop=(i == NWARM - 1))
            wstage = stats.tile([P, 4], F32, tag="wstage")
            nc.scalar.copy(wstage[:], wps[:, 0:4])

            # ---- input DMA stream (SP HWDGE queue, in this order).
            # ~0.6us serialized issue each -> keep the count minimal. ----
            for mi in (0, 1):
                t = work.tile([P, N_DCHUNK, P], FP16, tag="xT", bufs=4)
                nc.sync.dma_start(t[:], xTs_d[:, mi, :, :])
                xT0_3[mi] = t
            for j in range(N_YB):
                nc.sync.dma_start(yb[j][:], yT_src[j])
            for mi in (2, 3):
                t = work.tile([P, N_DCHUNK, P], FP16, tag="xT", bufs=4)
                nc.sync.dma_start(t[:], xTs_d[:, mi, :, :])
                xT0_3[mi] = t
            x16g[0] = xgp.tile([P, 4, D], FP16, tag="x16g", name="x16g0")
            nc.sync.dma_start(x16g[0][:], x16_src[0])
            # warm readback; its sem-wait sits behind the opening loads so
            # it never delays them.
            nc.sync.dma_start(warm_d, wstage[:])
            for g in (1, 2, 3):
                xTg[g] = xgp.tile([P, 4, N_DCHUNK, P], FP16, tag="xTg",
                                  name=f"xTg{g}")
                nc.sync.dma_start(xTg[g][:], xTs_d[:, 4 * g:4 * g + 4, :, :])
                x16g[g] = xgp.tile([P, 4, D], FP16, tag="x16g", name=f"x16g{g}")
                nc.sync.dma_start(x16g[g][:], x16_src[g])

            # ---- opening: 8 interleaved accumulation chains (m-tiles 0/1
            # x ns 0..3) fed k-step by k-step as each yb tile arrives;
            # tile 0's steps lead so its softmax can free banks early. ----
            chain = {(mi, ns): epsp.tile([P, NSLICE], F32, tag="e", name="eps")
                     for mi in (0, 1) for ns in range(N_NSL)}
            for j in range(N_YB):
                for mi in (0, 1):
                    for kk in range(YKB):
                        k = YKB * j + kk
                        for ns in range(N_NSL):
                            nc.tensor.matmul(
                                chain[(mi, ns)][:], get_xT(mi, k),
                                yb[j][:, kk, ns * NSLICE:(ns + 1) * NSLICE],
                                start=(k == 0), stop=(k == N_DCHUNK - 1),
                            )
            emit_softmax(0, [chain[(0, ns)] for ns in range(N_NSL)])
            emit_softmax(1, [chain[(1, ns)] for ns in range(N_NSL)])

            # ---- steady state ----
            for mi in range(2, N_MTILES):
                eps = [emit_mm1_group(mi, ns) for ns in range(N_NSL)]
                emit_softmax(mi, eps)

            # ---- mm2: accumulate all 16 p.T @ xs contributions per out
            # tile; DVE and ACT stage one 256-col half each in parallel,
            # then a single DMA writes the tile. ----
            HS = NSLICE // 2
            for nch in range(N_NCHUNK):
                for dh in range(N_DHALF):
                    ops = epsp.tile([P, NSLICE], F32, tag="e", name="ops")
                    for mi in range(N_MTILES):
                        nc.tensor.matmul(
                            ops[:],
                            pT[mi][:, nch * P:(nch + 1) * P],
                            xs[mi][:, dh * NSLICE:(dh + 1) * NSLICE],
                            start=(mi == 0), stop=(mi == N_MTILES - 1),
                        )
                    stg = work.tile([P, NSLICE], F32, tag="ostg", bufs=4)
                    nc.vector.tensor_copy(stg[:, 0:HS], ops[:, 0:HS])
                    nc.scalar.copy(stg[:, HS:NSLICE], ops[:, HS:NSLICE])
                    nc.sync.dma_start(
                        out_d[nch * P:(nch + 1) * P,
                              dh * NSLICE:(dh + 1) * NSLICE], stg[:])

    nc.compile()
    return nc


_NC_CACHE = {}


def _get_nc():
    if "nc" not in _NC_CACHE:
        _NC_CACHE["nc"] = _build_nc()
    return _NC_CACHE["nc"]


def _host_inputs(x_b: np.ndarray, y_b: np.ndarray) -> dict:
    x16 = x_b.astype(np.float16)
    xTs = np.ascontiguousarray(
        x16.reshape(N_MTILES, P, N_DCHUNK, P).transpose(3, 0, 2, 1))
    return {
        "xTs": xTs,
        "x16": np.ascontiguousarray(x16),
        "yT": np.ascontiguousarray(y_b.astype(np.float16).T),
    }


def kernel(x: np.ndarray, y: np.ndarray) -> np.ndarray:
    assert x.shape == (B, M, D) and y.shape == (B, N, D)
    nc = _get_nc()
    in_maps = [_host_inputs(x[b], y[b]) for b in range(B)]
    res = bass_utils.run_bass_kernel_spmd(nc, in_maps, core_ids=list(range(B)))
    return np.stack([res.results[b]["out"] for b in range(B)], axis=0)


# revision 10
# speedup vs baseline: 1.0346x; 1.0215x over previous
"""InnerAttention kernel for 8 Trainium2 NeuronCores.

Computes, per batch b:
    e = x[b] @ y[b].T          [M, N]
    p = softmax(e, axis=-1)    (over n)
    out[b] = p.T @ x[b]        [N, D]

Sharding: data-parallel over batch (B=8 -> one batch per core). Full inputs
in, full output out. The host-side wrapper pre-casts to fp16 and pre-builds
device-friendly layouts with numpy so every DMA is 128 partitions x >=2KB
contiguous per partition:
  xTs [P, 16, 8, P] fp16  (xTs[p,mi,k,c] = x[128mi+c, 128k+p]; mm1
                           stationary chunks, loaded 1 m-tile per DMA for
                           tiles 0-3 and 4 m-tiles per DMA for 4-15)
  x16 [M, D] fp16         (x natural; mm2 moving after 1/s scaling; loaded
                           4 m-tiles per DMA)
  yT  [D, N] fp16         (y transposed, mm1 moving; loaded as 4 DMAs of
                           [128, 2, 2048] = 2 d-chunks each)

HWDGE descriptor generation costs ~0.6us per dma_start and is serialized,
so the input stream is exactly 16 DMA instructions.

Per-core (M=N=2048, D=1024, P=128):
  mm1 opening: 8 PSUM accumulation chains (m-tiles 0/1 x 4 n-slices)
       interleaved k-step by k-step behind the 4 yT DMAs, so the PE
       saturates as soon as data drips in (PSUM accumulate is per-element
       has_written state, so interleaved chains to different banks are
       fine). Dummy matmuls before that burn the 1.2->2.4GHz clock ramp.
  softmax: DVE row-max over PSUM, ACT exp (bias=-max) -> p fp16 in SBUF,
       accum_out row-sum; 1/sum folded into xs = x16 * (1/s) (fp16).
  mm2: per (n-chunk 128, d-half 512) out tile: accumulate all 16
       p.T @ xs contributions in one PSUM bank; DVE and ACT copy one
       256-col half each in parallel into one staging tile, single DMA out.
"""

import numpy as np

import concourse.bacc as bacc
import concourse.mybir as mybir
import concourse.tile as tile
from concourse import bass_utils

B, M, N, D = 8, 2048, 2048, 1024
P = 128
NSLICE = 512
N_MTILES = M // P     # 16
N_DCHUNK = D // P     # 8
N_NSL = N // NSLICE   # 4
N_NCHUNK = N // P     # 16
N_DHALF = D // NSLICE  # 2
NWARM = 40
YKB = 2               # d-chunks per yT DMA
N_YB = N_DCHUNK // YKB  # 4

F32 = mybir.dt.float32
FP16 = mybir.dt.float16
AX = mybir.AxisListType.X
EXP = mybir.ActivationFunctionType.Exp


def _build_nc():
    nc = bacc.Bacc("TRN2", target_bir_lowering=False, debug=False)
    xTs_d = nc.dram_tensor("xTs", [P, N_MTILES, N_DCHUNK, P], FP16,
                           kind="ExternalInput").ap()
    x16_d = nc.dram_tensor("x16", [M, D], FP16, kind="ExternalInput").ap()
    yT_d = nc.dram_tensor("yT", [D, N], FP16, kind="ExternalInput").ap()
    out_d = nc.dram_tensor("out", [N, D], F32, kind="ExternalOutput").ap()
    warm_d = nc.dram_tensor("warm_o", [P, 4], F32, kind="ExternalOutput").ap()

    with tile.TileContext(nc) as tc:
        with (
            tc.tile_pool(name="yTp", bufs=1) as yTp,
            tc.tile_pool(name="pP", bufs=1) as pPp,
            tc.tile_pool(name="xsP", bufs=1) as xsPp,
            tc.tile_pool(name="xg", bufs=3) as xgp,
            tc.tile_pool(name="work", bufs=2) as work,
            tc.tile_pool(name="stats", bufs=3) as stats,
            tc.tile_pool(name="eps", bufs=8, space="PSUM") as epsp,
        ):
            # yT big tiles: yb[j][p, kk, n] = y[n, 128*(2j+kk)+p]
            yb = [yTp.tile([P, YKB, N], FP16, tag=f"yb{j}", name=f"yb{j}")
                  for j in range(N_YB)]
            pT = [pPp.tile([P, N], FP16, tag=f"prob{mi}", name=f"prob{mi}")
                  for mi in range(N_MTILES)]
            xs = [xsPp.tile([P, D], FP16, tag=f"xsc{mi}", name=f"xsc{mi}")
                  for mi in range(N_MTILES)]

            xT0_3 = {}
            xTg = {}
            x16g = {}
            yT_src = yT_d.rearrange("(a b p) n -> a p b n", b=YKB, p=P)
            x16_src = x16_d.rearrange("(g t p) d -> g p t d", t=4, p=P)

            def get_xT(mi, k):
                if mi < 4:
                    return xT0_3[mi][:, k, :]
                return xTg[mi // 4][:, mi % 4, k, :]

            def get_x16(mi):
                return x16g[mi // 4][:, mi % 4, :]

            def emit_mm1_group(mi, ns):
                ep = epsp.tile([P, NSLICE], F32, tag="e", name="eps")
                for k in range(N_DCHUNK):
                    nc.tensor.matmul(
                        ep[:], get_xT(mi, k),
                        yb[k // YKB][:, k % YKB, ns * NSLICE:(ns + 1) * NSLICE],
                        start=(k == 0), stop=(k == N_DCHUNK - 1),
                    )
                return ep

            rinv_of = {}

            def emit_scale(mi):
                # xs[mi] = x16[mi] * (1/sum); deferred one tile so a late
                # x16 DMA never blocks the DVE rmax -> ACT exp chain that
                # frees PSUM banks.
                nc.vector.tensor_scalar_mul(
                    xs[mi][:], get_x16(mi), rinv_of.pop(mi)[:])

            def emit_softmax(mi, eps):
                rmax4 = stats.tile([P, N_NSL], F32, tag="rmax4")
                for ns in range(N_NSL):
                    nc.vector.reduce_max(rmax4[:, ns:ns + 1], eps[ns][:], axis=AX)
                negmax = stats.tile([P, 1], F32, tag="negmax")
                rmax = stats.tile([P, 1], F32, tag="rmax")
                nc.vector.reduce_max(rmax[:], rmax4[:], axis=AX)
                nc.vector.tensor_scalar_mul(negmax[:], rmax[:], -1.0)

                s4 = stats.tile([P, N_NSL], F32, tag="s4")
                for ns in range(N_NSL):
                    c0 = ns * NSLICE
                    nc.scalar.activation(
                        pT[mi][:, c0:c0 + NSLICE], eps[ns][:], EXP,
                        bias=negmax[:], accum_out=s4[:, ns:ns + 1],
                    )
                ssum = stats.tile([P, 1], F32, tag="ssum")
                nc.vector.reduce_sum(ssum[:], s4[:], axis=AX)
                rinv = stats.tile([P, 1], F32, tag="rinv")
                nc.vector.reciprocal(rinv[:], ssum[:])
                rinv_of[mi] = rinv
                if mi > 0:
                    emit_scale(mi - 1)

            # ---- clock warmup: the PE ramps 1.2->2.4GHz only after ~3us
            # of continuous work; burn the ramp on dummy matmuls during
            # the initial DMA fill. ----
            wsrc = work.tile([P, P], FP16, tag="wsrc", bufs=1)
            nc.vector.memset(wsrc[:], 0.0)
            wps = epsp.tile([P, NSLICE], F32, tag="e", name="warmps")
            for i in range(NWARM):
                nc.tensor.matmul(wps[:, 0:P], wsrc[:], wsrc[:],
                                 start=(i == 0), stop=(i == NWARM - 1))
            wstage = stats.tile([P, 4], F32, tag="wstage")
            nc.scalar.copy(wstage[:], wps[:, 0:4])

            # ---- input DMA stream (SP HWDGE queue; data arrives strictly
            # FIFO at ~345GB/s and issue is ~0.6-2us serialized each, so
            # both the count and the order are load-bearing). ----
            def emit_xT03(mi):
                t = work.tile([P, N_DCHUNK, P], FP16, tag="xT", bufs=4)
                nc.sync.dma_start(t[:], xTs_d[:, mi, :, :])
                xT0_3[mi] = t

            def emit_xTg(g):
                xTg[g] = xgp.tile([P, 4, N_DCHUNK, P], FP16, tag="xTg",
                                  name=f"xTg{g}")
                nc.sync.dma_start(xTg[g][:], xTs_d[:, 4 * g:4 * g + 4, :, :])

            def emit_x16g(g):
                x16g[g] = xgp.tile([P, 4, D], FP16, tag="x16g", name=f"x16g{g}")
                nc.sync.dma_start(x16g[g][:], x16_src[g])

            nc.sync.dma_start(yb[0][:], yT_src[0])
            emit_xT03(0)
            emit_xT03(1)
            for j in range(1, N_YB):
                nc.sync.dma_start(yb[j][:], yT_src[j])
            emit_xT03(2)
            emit_xT03(3)
            emit_xTg(1)
            emit_x16g(0)
            emit_xTg(2)
            emit_x16g(1)
            emit_xTg(3)
            emit_x16g(2)
            emit_x16g(3)
            # warm readback last; its sem-wait never delays real loads.
            nc.sync.dma_start(warm_d, wstage[:])

            # ---- opening: 8 interleaved accumulation chains (m-tiles 0/1
            # x ns 0..3) fed k-step by k-step as each yb tile arrives;
            # tile 0's steps lead so its softmax can free banks early. ----
            chain = {(mi, ns): epsp.tile([P, NSLICE], F32, tag="e", name="eps")
                     for mi in (0, 1) for ns in range(N_NSL)}
            for j in range(N_YB):
                for mi in (0, 1):
                    for kk in range(YKB):
                        k = YKB * j + kk
                        for ns in range(N_NSL):
                            nc.tensor.matmul(
                                chain[(mi, ns)][:], get_xT(mi, k),
                                yb[j][:, kk, ns * NSLICE:(ns + 1) * NSLICE],
                                start=(k == 0), stop=(k == N_DCHUNK - 1),
                            )
            emit_softmax(0, [chain[(0, ns)] for ns in range(N_NSL)])
            emit_softmax(1, [chain[(1, ns)] for ns in range(N_NSL)])

            # ---- steady state ----
            for mi in range(2, N_MTILES):
                eps = [emit_mm1_group(mi, ns) for ns in range(N_NSL)]
                emit_softmax(mi, eps)
            emit_scale(N_MTILES - 1)

            # ---- mm2: accumulate all 16 p.T @ xs contributions per out
            # tile; DVE and ACT stage one 256-col half each in parallel,
            # then a single DMA writes the tile. ----
            HS = NSLICE // 2
            for nch in range(N_NCHUNK):
                for dh in range(N_DHALF):
                    ops = epsp.tile([P, NSLICE], F32, tag="e", name="ops")
                    for mi in range(N_MTILES):
                        nc.tensor.matmul(
                            ops[:],
                            pT[mi][:, nch * P:(nch + 1) * P],
                            xs[mi][:, dh * NSLICE:(dh + 1) * NSLICE],
                            start=(mi == 0), stop=(mi == N_MTILES - 1),
                        )
                    stg = work.tile([P, NSLICE], F32, tag="ostg", bufs=4)
                    nc.vector.tensor_copy(stg[:, 0:HS], ops[:, 0:HS])
                    nc.scalar.copy(stg[:, HS:NSLICE], ops[:, HS:NSLICE])
                    nc.sync.dma_start(
                        out_d[nch * P:(nch + 1) * P,
                              dh * NSLICE:(dh + 1) * NSLICE], stg[:])

    nc.compile()
    return nc


_NC_CACHE = {}


def _get_nc():
    if "nc" not in _NC_CACHE:
        _NC_CACHE["nc"] = _build_nc()
    return _NC_CACHE["nc"]


def _host_inputs(x_b: np.ndarray, y_b: np.ndarray) -> dict:
    x16 = x_b.astype(np.float16)
    xTs = np.ascontiguousarray(
        x16.reshape(N_MTILES, P, N_DCHUNK, P).transpose(3, 0, 2, 1))
    return {
        "xTs": xTs,
        "x16": np.ascontiguousarray(x16),
        "yT": np.ascontiguousarray(y_b.astype(np.float16).T),
    }


def kernel(x: np.ndarray, y: np.ndarray) -> np.ndarray:
    assert x.shape == (B, M, D) and y.shape == (B, N, D)
    nc = _get_nc()
    in_maps = [_host_inputs(x[b], y[b]) for b in range(B)]
    res = bass_utils.run_bass_kernel_spmd(nc, in_maps, core_ids=list(range(B)))
    return np.stack([res.results[b]["out"] for b in range(B)], axis=0)


# revision 12
# speedup vs baseline: 1.0357x; 1.0010x over previous
"""InnerAttention kernel for 8 Trainium2 NeuronCores.

Computes, per batch b:
    e = x[b] @ y[b].T          [M, N]
    p = softmax(e, axis=-1)    (over n)
    out[b] = p.T @ x[b]        [N, D]

Sharding: data-parallel over batch (B=8 -> one batch per core). Full inputs
in, full output out. The host-side wrapper pre-casts to fp16 and pre-builds
device-friendly layouts with numpy so every DMA is 128 partitions x >=2KB
contiguous per partition:
  xTs [P, 16, 8, P] fp16  (xTs[p,mi,k,c] = x[128mi+c, 128k+p]; mm1
                           stationary chunks, loaded 1 m-tile per DMA for
                           tiles 0-3 and 4 m-tiles per DMA for 4-15)
  x16 [M, D] fp16         (x natural; mm2 moving after 1/s scaling; loaded
                           4 m-tiles per DMA)
  yT  [D, N] fp16         (y transposed, mm1 moving; loaded as 4 DMAs of
                           [128, 2, 2048] = 2 d-chunks each)

HWDGE descriptor generation costs ~0.6us per dma_start and is serialized,
so the input stream is exactly 16 DMA instructions.

Per-core (M=N=2048, D=1024, P=128):
  mm1 opening: 8 PSUM accumulation chains (m-tiles 0/1 x 4 n-slices)
       interleaved k-step by k-step behind the 4 yT DMAs, so the PE
       saturates as soon as data drips in (PSUM accumulate is per-element
       has_written state, so interleaved chains to different banks are
       fine). Dummy matmuls before that burn the 1.2->2.4GHz clock ramp.
  softmax: DVE row-max over PSUM, ACT exp (bias=-max) -> p fp16 in SBUF,
       accum_out row-sum; 1/sum folded into xs = x16 * (1/s) (fp16).
  mm2: per (n-chunk 128, d-half 512) out tile: accumulate all 16
       p.T @ xs contributions in one PSUM bank; DVE and ACT copy one
       256-col half each in parallel into one staging tile, single DMA out.
"""

import numpy as np

import concourse.bacc as bacc
import concourse.mybir as mybir
import concourse.tile as tile
from concourse import bass_utils

B, M, N, D = 8, 2048, 2048, 1024
P = 128
NSLICE = 512
N_MTILES = M // P     # 16
N_DCHUNK = D // P     # 8
N_NSL = N // NSLICE   # 4
N_NCHUNK = N // P     # 16
N_DHALF = D // NSLICE  # 2
NWARM = 40
YKB = 2               # d-chunks per yT DMA
N_YB = N_DCHUNK // YKB  # 4

F32 = mybir.dt.float32
FP16 = mybir.dt.float16
AX = mybir.AxisListType.X
EXP = mybir.ActivationFunctionType.Exp


def _build_nc():
    nc = bacc.Bacc("TRN2", target_bir_lowering=False, debug=False)
    xTs_d = nc.dram_tensor("xTs", [P, N_MTILES, N_DCHUNK, P], FP16,
                           kind="ExternalInput").ap()
    x16_d = nc.dram_tensor("x16", [M, D], FP16, kind="ExternalInput").ap()
    yT_d = nc.dram_tensor("yT", [D, N], FP16, kind="ExternalInput").ap()
    out_d = nc.dram_tensor("out", [N, D], F32, kind="ExternalOutput").ap()
    warm_d = nc.dram_tensor("warm_o", [P, 4], F32, kind="ExternalOutput").ap()

    with tile.TileContext(nc) as tc:
        with (
            tc.tile_pool(name="yTp", bufs=1) as yTp,
            tc.tile_pool(name="pP", bufs=1) as pPp,
            tc.tile_pool(name="xsP", bufs=1) as xsPp,
            tc.tile_pool(name="xg", bufs=3) as xgp,
            tc.tile_pool(name="work", bufs=2) as work,
            tc.tile_pool(name="stats", bufs=3) as stats,
            tc.tile_pool(name="eps", bufs=8, space="PSUM") as epsp,
        ):
            # yT big tiles: yb[j][p, kk, n] = y[n, 128*(2j+kk)+p]
            yb = [yTp.tile([P, YKB, N], FP16, tag=f"yb{j}", name=f"yb{j}")
                  for j in range(N_YB)]
            pT = [pPp.tile([P, N], FP16, tag=f"prob{mi}", name=f"prob{mi}")
                  for mi in range(N_MTILES)]
            xs = [xsPp.tile([P, D], FP16, tag=f"xsc{mi}", name=f"xsc{mi}")
                  for mi in range(N_MTILES)]

            xT0_3 = {}
            xTg = {}
            x16g = {}
            yT_src = yT_d.rearrange("(a b p) n -> a p b n", b=YKB, p=P)
            x16_src = x16_d.rearrange("(g t p) d -> g p t d", t=4, p=P)

            def get_xT(mi, k):
                if mi < 4:
                    return xT0_3[mi][:, k, :]
                return xTg[mi // 4][:, mi % 4, k, :]

            def get_x16(mi):
                return x16g[mi // 4][:, mi % 4, :]

            def emit_mm1_group(mi, ns):
                ep = epsp.tile([P, NSLICE], F32, tag="e", name="eps")
                for k in range(N_DCHUNK):
                    nc.tensor.matmul(
                        ep[:], get_xT(mi, k),
                        yb[k // YKB][:, k % YKB, ns * NSLICE:(ns + 1) * NSLICE],
                        start=(k == 0), stop=(k == N_DCHUNK - 1),
                    )
                return ep

            rinv_of = {}

            def emit_scale(mi):
                # xs[mi] = x16[mi] * (1/sum); deferred one tile so a late
                # x16 DMA never blocks the DVE rmax -> ACT exp chain that
                # frees PSUM banks.
                nc.vector.tensor_scalar_mul(
                    xs[mi][:], get_x16(mi), rinv_of.pop(mi)[:])

            def emit_softmax(mi, eps):
                rmax4 = stats.tile([P, N_NSL], F32, tag="rmax4")
                for ns in range(N_NSL):
                    nc.vector.reduce_max(rmax4[:, ns:ns + 1], eps[ns][:], axis=AX)
                negmax = stats.tile([P, 1], F32, tag="negmax")
                rmax = stats.tile([P, 1], F32, tag="rmax")
                nc.vector.reduce_max(rmax[:], rmax4[:], axis=AX)
                nc.vector.tensor_scalar_mul(negmax[:], rmax[:], -1.0)

                s4 = stats.tile([P, N_NSL], F32, tag="s4")
                for ns in range(N_NSL):
                    c0 = ns * NSLICE
                    nc.scalar.activation(
                        pT[mi][:, c0:c0 + NSLICE], eps[ns][:], EXP,
                        bias=negmax[:], accum_out=s4[:, ns:ns + 1],
                    )
                ssum = stats.tile([P, 1], F32, tag="ssum")
                nc.vector.reduce_sum(ssum[:], s4[:], axis=AX)
                rinv = stats.tile([P, 1], F32, tag="rinv")
                nc.vector.reciprocal(rinv[:], ssum[:])
                rinv_of[mi] = rinv
                if mi > 0:
                    emit_scale(mi - 1)

            # ---- clock warmup: the PE ramps 1.2->2.4GHz only after ~3us
            # of continuous work; burn the ramp on dummy matmuls during
            # the initial DMA fill. ----
            wsrc = work.tile([P, P], FP16, tag="wsrc", bufs=1)
            nc.vector.memset(wsrc[:], 0.0)
            wps = epsp.tile([P, NSLICE], F32, tag="e", name="warmps")
            for i in range(NWARM):
                nc.tensor.matmul(wps[:, 0:P], wsrc[:], wsrc[:],
                                 start=(i == 0), stop=(i == NWARM - 1))
            wstage = stats.tile([P, 4], F32, tag="wstage")
            nc.scalar.copy(wstage[:], wps[:, 0:4])

            # ---- input DMA stream (SP HWDGE queue; data arrives strictly
            # FIFO at ~345GB/s and issue is ~0.6-2us serialized each, so
            # both the count and the order are load-bearing). ----
            def emit_xT03(mi):
                t = work.tile([P, N_DCHUNK, P], FP16, tag="xT", bufs=4)
                nc.sync.dma_start(t[:], xTs_d[:, mi, :, :])
                xT0_3[mi] = t

            def emit_xTg(g):
                xTg[g] = xgp.tile([P, 4, N_DCHUNK, P], FP16, tag="xTg",
                                  name=f"xTg{g}")
                nc.sync.dma_start(xTg[g][:], xTs_d[:, 4 * g:4 * g + 4, :, :])

            def emit_x16g(g):
                x16g[g] = xgp.tile([P, 4, D], FP16, tag="x16g", name=f"x16g{g}")
                nc.sync.dma_start(x16g[g][:], x16_src[g])

            nc.sync.dma_start(yb[0][:], yT_src[0])
            emit_xT03(0)
            emit_xT03(1)
            for j in range(1, N_YB):
                nc.sync.dma_start(yb[j][:], yT_src[j])
            emit_xT03(2)
            emit_xT03(3)
            emit_xTg(1)
            emit_x16g(0)
            emit_xTg(2)
            emit_x16g(1)
            emit_xTg(3)
            emit_x16g(2)
            emit_x16g(3)
            # warm readback last; its sem-wait never delays real loads.
            nc.sync.dma_start(warm_d, wstage[:])

            # ---- opening: 6 interleaved accumulation chains (m-tiles 0/1
            # x ns 0..2) fed k-step by k-step as each yb tile arrives; the
            # ns=3 groups run as plain groups afterwards on resident data
            # while exp(0,*) frees PSUM banks for the steady state (with 8
            # chains, all stops cluster and G(2,*) stalls on bank WAW). ----
            NSC = N_NSL - 1
            chain = {(mi, ns): epsp.tile([P, NSLICE], F32, tag="e", name="eps")
                     for mi in (0, 1) for ns in range(NSC)}
            for j in range(N_YB):
                for mi in (0, 1):
                    for kk in range(YKB):
                        k = YKB * j + kk
                        for ns in range(NSC):
                            nc.tensor.matmul(
                                chain[(mi, ns)][:], get_xT(mi, k),
                                yb[j][:, kk, ns * NSLICE:(ns + 1) * NSLICE],
                                start=(k == 0), stop=(k == N_DCHUNK - 1),
                            )
            ep03 = emit_mm1_group(0, N_NSL - 1)
            ep13 = emit_mm1_group(1, N_NSL - 1)
            emit_softmax(0, [chain[(0, ns)] for ns in range(NSC)] + [ep03])
            emit_softmax(1, [chain[(1, ns)] for ns in range(NSC)] + [ep13])

            # ---- steady state ----
            for mi in range(2, N_MTILES):
                eps = [emit_mm1_group(mi, ns) for ns in range(N_NSL)]
                emit_softmax(mi, eps)
            emit_scale(N_MTILES - 1)

            # ---- mm2: accumulate all 16 p.T @ xs contributions per out
            # tile; DVE and ACT stage one 256-col half each in parallel,
            # then a single DMA writes the tile. ----
            HS = NSLICE // 2
            for nch in range(N_NCHUNK):
                for dh in range(N_DHALF):
                    ops = epsp.tile([P, NSLICE], F32, tag="e", name="ops")
                    for mi in range(N_MTILES):
                        nc.tensor.matmul(
                            ops[:],
                            pT[mi][:, nch * P:(nch + 1) * P],
                            xs[mi][:, dh * NSLICE:(dh + 1) * NSLICE],
                            start=(mi == 0), stop=(mi == N_MTILES - 1),
                        )
                    # separate half tiles: one staging tile would serialize
                    # the DVE and ACT writers (framework WAW ordering)
                    rows = slice(nch * P, (nch + 1) * P)
                    c0 = dh * NSLICE
                    hA = work.tile([P, HS], F32, tag="ostgA", bufs=4)
                    hB = work.tile([P, HS], F32, tag="ostgB", bufs=4)
                    nc.vector.tensor_copy(hA[:], ops[:, 0:HS])
                    nc.scalar.copy(hB[:], ops[:, HS:NSLICE])
                    nc.sync.dma_start(out_d[rows, c0:c0 + HS], hA[:])
                    nc.scalar.dma_start(out_d[rows, c0 + HS:c0 + NSLICE], hB[:])

    nc.compile()
    return nc


_NC_CACHE = {}


def _get_nc():
    if "nc" not in _NC_CACHE:
        _NC_CACHE["nc"] = _build_nc()
    return _NC_CACHE["nc"]


def _host_inputs(x_b: np.ndarray, y_b: np.ndarray) -> dict:
    x16 = x_b.astype(np.float16)
    xTs = np.ascontiguousarray(
        x16.reshape(N_MTILES, P, N_DCHUNK, P).transpose(3, 0, 2, 1))
    return {
        "xTs": xTs,
        "x16": np.ascontiguousarray(x16),
        "yT": np.ascontiguousarray(y_b.astype(np.float16).T),
    }


def kernel(x: np.ndarray, y: np.ndarray) -> np.ndarray:
    assert x.shape == (B, M, D) and y.shape == (B, N, D)
    nc = _get_nc()
    in_maps = [_host_inputs(x[b], y[b]) for b in range(B)]
    res = bass_utils.run_bass_kernel_spmd(nc, in_maps, core_ids=list(range(B)))
    return np.stack([res.results[b]["out"] for b in range(B)], axis=0)


# revision 19
# speedup vs baseline: 1.0404x; 1.0045x over previous
"""InnerAttention kernel for 8 Trainium2 NeuronCores.

Computes, per batch b:
    e = x[b] @ y[b].T          [M, N]
    p = softmax(e, axis=-1)    (over n)
    out[b] = p.T @ x[b]        [N, D]

Sharding: data-parallel over batch (B=8 -> one batch per core). Full inputs
in, full output out. The host-side wrapper pre-casts to fp16 and pre-builds
device-friendly layouts with numpy so every DMA is 128 partitions x >=2KB
contiguous per partition:
  xTs [P, 16, 8, P] fp16  (xTs[p,mi,k,c] = x[128mi+c, 128k+p]; mm1
                           stationary chunks, loaded 1 m-tile per DMA for
                           tiles 0-3 and 4 m-tiles per DMA for 4-15)
  x16 [M, D] fp16         (x natural; mm2 moving after 1/s scaling; loaded
                           4 m-tiles per DMA)
  yT  [D, N] fp16         (y transposed, mm1 moving; loaded as 4 DMAs of
                           [128, 2, 2048] = 2 d-chunks each)

HWDGE descriptor generation costs ~0.6us per dma_start and is serialized,
so the input stream is exactly 16 DMA instructions.

Per-core (M=N=2048, D=1024, P=128):
  mm1 opening: 8 PSUM accumulation chains (m-tiles 0/1 x 4 n-slices)
       interleaved k-step by k-step behind the 4 yT DMAs, so the PE
       saturates as soon as data drips in (PSUM accumulate is per-element
       has_written state, so interleaved chains to different banks are
       fine). Dummy matmuls before that burn the 1.2->2.4GHz clock ramp.
  softmax: DVE row-max over PSUM, ACT exp (bias=-max) -> p fp16 in SBUF,
       accum_out row-sum; 1/sum folded into xs = x16 * (1/s) (fp16).
  mm2: per (n-chunk 128, d-half 512) out tile: accumulate all 16
       p.T @ xs contributions in one PSUM bank; DVE and ACT copy one
       256-col half each in parallel into one staging tile, single DMA out.
"""

import numpy as np

import concourse.bacc as bacc
import concourse.mybir as mybir
import concourse.tile as tile
from concourse import bass_utils

B, M, N, D = 8, 2048, 2048, 1024
P = 128
NSLICE = 512
N_MTILES = M // P     # 16
N_DCHUNK = D // P     # 8
N_NSL = N // NSLICE   # 4
N_NCHUNK = N // P     # 16
N_DHALF = D // NSLICE  # 2
NWARM = 40
YKB = 2               # d-chunks per yT DMA
N_YB = N_DCHUNK // YKB  # 4
NSC = N_NSL - 1       # n-slices covered by the opening interleaved chains

F32 = mybir.dt.float32
FP16 = mybir.dt.float16
AX = mybir.AxisListType.X
EXP = mybir.ActivationFunctionType.Exp


def _build_nc():
    nc = bacc.Bacc("TRN2", target_bir_lowering=False, debug=False)
    xTs_d = nc.dram_tensor("xTs", [P, N_MTILES, N_DCHUNK, P], FP16,
                           kind="ExternalInput").ap()
    x16_d = nc.dram_tensor("x16", [M, D], FP16, kind="ExternalInput").ap()
    yT_d = nc.dram_tensor("yT", [D, N], FP16, kind="ExternalInput").ap()
    out_d = nc.dram_tensor("out", [N, D], F32, kind="ExternalOutput").ap()
    warm_d = nc.dram_tensor("warm_o", [P, 4], F32, kind="ExternalOutput").ap()

    with tile.TileContext(nc) as tc:
        with (
            tc.tile_pool(name="yTp", bufs=1) as yTp,
            tc.tile_pool(name="pP", bufs=1) as pPp,
            tc.tile_pool(name="xsP", bufs=1) as xsPp,
            tc.tile_pool(name="xg", bufs=3) as xgp,
            tc.tile_pool(name="work", bufs=2) as work,
            tc.tile_pool(name="stats", bufs=3) as stats,
            tc.tile_pool(name="eps", bufs=8, space="PSUM") as epsp,
        ):
            # yT big tiles: yb[j][p, kk, n] = y[n, 128*(2j+kk)+p]
            yb = [yTp.tile([P, YKB, N], FP16, tag=f"yb{j}", name=f"yb{j}")
                  for j in range(N_YB)]
            pT = [pPp.tile([P, N], FP16, tag=f"prob{mi}", name=f"prob{mi}")
                  for mi in range(N_MTILES)]
            xs = [xsPp.tile([P, D], FP16, tag=f"xsc{mi}", name=f"xsc{mi}")
                  for mi in range(N_MTILES)]

            xT0_3 = {}
            xTg = {}
            x16g = {}
            yT_src = yT_d.rearrange("(a b p) n -> a p b n", b=YKB, p=P)
            x16_src = x16_d.rearrange("(g t p) d -> g p t d", t=4, p=P)

            def get_xT(mi, k):
                if mi < 4:
                    return xT0_3[mi][:, k, :]
                return xTg[mi // 4][:, mi % 4, k, :]

            def get_x16(mi):
                return x16g[mi // 4][:, mi % 4, :]

            def emit_mm1_group(mi, ns):
                ep = epsp.tile([P, NSLICE], F32, tag="e", name="eps")
                for k in range(N_DCHUNK):
                    nc.tensor.matmul(
                        ep[:], get_xT(mi, k),
                        yb[k // YKB][:, k % YKB, ns * NSLICE:(ns + 1) * NSLICE],
                        start=(k == 0), stop=(k == N_DCHUNK - 1),
                    )
                return ep

            rinv_of = {}
            corr_of = {}
            xsA = {}

            def emit_scale(mi):
                # xs[mi] = x16[mi] * (1/sum); deferred so a late x16 DMA
                # never blocks the DVE rmax -> ACT exp chain that frees
                # PSUM banks. Opening tiles (block-max softmax) also get
                # xsA[mi] = xs[mi] * corr for the ns0..2 column blocks.
                rinv = rinv_of.pop(mi)
                if mi in corr_of:
                    sc = stats.tile([P, 1], F32, tag="scA")
                    nc.vector.tensor_mul(sc[:], rinv[:], corr_of.pop(mi)[:])
                    xsA[mi] = xsPp.tile([P, D], FP16, tag=f"xsA{mi}",
                                        name=f"xsA{mi}")
                    nc.vector.tensor_scalar_mul(xsA[mi][:], get_x16(mi), sc[:])
                nc.vector.tensor_scalar_mul(xs[mi][:], get_x16(mi), rinv[:])

            def emit_softmax(mi, eps):
                rmax4 = stats.tile([P, N_NSL], F32, tag="rmax4")
                for ns in range(N_NSL):
                    nc.vector.reduce_max(rmax4[:, ns:ns + 1], eps[ns][:], axis=AX)
                negmax = stats.tile([P, 1], F32, tag="negmax")
                rmax = stats.tile([P, 1], F32, tag="rmax")
                nc.vector.reduce_max(rmax[:], rmax4[:], axis=AX)
                nc.vector.tensor_scalar_mul(negmax[:], rmax[:], -1.0)

                s4 = stats.tile([P, N_NSL], F32, tag="s4")
                for ns in range(N_NSL):
                    c0 = ns * NSLICE
                    nc.scalar.activation(
                        pT[mi][:, c0:c0 + NSLICE], eps[ns][:], EXP,
                        bias=negmax[:], accum_out=s4[:, ns:ns + 1],
                    )
                ssum = stats.tile([P, 1], F32, tag="ssum")
                nc.vector.reduce_sum(ssum[:], s4[:], axis=AX)
                rinv = stats.tile([P, 1], F32, tag="rinv")
                nc.vector.reciprocal(rinv[:], ssum[:])
                rinv_of[mi] = rinv
                if mi - 2 >= 0:
                    emit_scale(mi - 2)

            # --- block-max softmax for the opening tiles: exp the ns0..2
            # slices as soon as their chains stop, biased by the partial
            # max m' = max(ns0..2) (values <= 1, fp16-safe). The true row
            # max m folds in later via corr = exp(m' - m): the exact sum
            # is S = s012*corr + s3, and corr rides into mm2 through
            # xsA = xs*corr used for the ns0..2 n-chunks. This frees the
            # opening PSUM banks ~2.5us earlier than a full-row max. ---
            def emit_open_head(mi, eps012):
                rmax3 = stats.tile([P, NSC], F32, tag="rmax3")
                for ns in range(NSC):
                    nc.vector.reduce_max(rmax3[:, ns:ns + 1], eps012[ns][:],
                                         axis=AX)
                mprime = stats.tile([P, 1], F32, tag="mprime")
                nc.vector.reduce_max(mprime[:], rmax3[:], axis=AX)
                negmp = stats.tile([P, 1], F32, tag="negmp")
                nc.vector.tensor_scalar_mul(negmp[:], mprime[:], -1.0)
                s4 = stats.tile([P, N_NSL], F32, tag="s4")
                for ns in range(NSC):
                    nc.scalar.activation(
                        pT[mi][:, ns * NSLICE:(ns + 1) * NSLICE],
                        eps012[ns][:], EXP,
                        bias=negmp[:], accum_out=s4[:, ns:ns + 1],
                    )
                return mprime, s4

            def emit_open_tail(mi, ep3, mprime, s4):
                r3 = stats.tile([P, 1], F32, tag="r3")
                nc.vector.reduce_max(r3[:], ep3[:], axis=AX)
                m = stats.tile([P, 1], F32, tag="mfull")
                nc.vector.tensor_max(m[:], r3[:], mprime[:])
                negm = stats.tile([P, 1], F32, tag="negm")
                nc.vector.tensor_scalar_mul(negm[:], m[:], -1.0)
                nc.scalar.activation(
                    pT[mi][:, NSC * NSLICE:], ep3[:], EXP,
                    bias=negm[:], accum_out=s4[:, NSC:NSC + 1],
                )
                corr = stats.tile([P, 1], F32, tag="corr")
                nc.scalar.activation(corr[:], mprime[:], EXP, bias=negm[:])
                s012 = stats.tile([P, 1], F32, tag="s012")
                nc.vector.reduce_sum(s012[:], s4[:, 0:NSC], axis=AX)
                sA = stats.tile([P, 1], F32, tag="sA")
                nc.vector.tensor_mul(sA[:], s012[:], corr[:])
                stot = stats.tile([P, 1], F32, tag="stot")
                nc.vector.tensor_add(stot[:], sA[:], s4[:, NSC:NSC + 1])
                rinv = stats.tile([P, 1], F32, tag="rinv")
                nc.vector.reciprocal(rinv[:], stot[:])
                rinv_of[mi] = rinv
                corr_of[mi] = corr

            # ---- clock warmup: the PE ramps 1.2->2.4GHz only after ~3us
            # of continuous work; burn the ramp on dummy matmuls during
            # the initial DMA fill. ----
            wsrc = work.tile([P, P], FP16, tag="wsrc", bufs=1)
            nc.vector.memset(wsrc[:], 0.0)
            wps = epsp.tile([P, NSLICE], F32, tag="e", name="warmps")
            for i in range(NWARM):
                nc.tensor.matmul(wps[:, 0:P], wsrc[:], wsrc[:],
                                 start=(i == 0), stop=(i == NWARM - 1))
            wstage = stats.tile([P, 4], F32, tag="wstage")
            nc.scalar.copy(wstage[:], wps[:, 0:4])

            # ---- input DMA stream (SP HWDGE queue; data arrives strictly
            # FIFO at ~345GB/s and issue is ~0.6-2us serialized each, so
            # both the count and the order are load-bearing). ----
            def emit_xT03(mi):
                t = work.tile([P, N_DCHUNK, P], FP16, tag="xT", bufs=4)
                nc.sync.dma_start(t[:], xTs_d[:, mi, :, :])
                xT0_3[mi] = t

            def emit_xTg(g):
                xTg[g] = xgp.tile([P, 4, N_DCHUNK, P], FP16, tag="xTg",
                                  name=f"xTg{g}")
                nc.sync.dma_start(xTg[g][:], xTs_d[:, 4 * g:4 * g + 4, :, :])

            def emit_x16g(g):
                x16g[g] = xgp.tile([P, 4, D], FP16, tag="x16g", name=f"x16g{g}")
                nc.sync.dma_start(x16g[g][:], x16_src[g])

            nc.sync.dma_start(yb[0][:], yT_src[0])
            emit_xT03(0)
            emit_xT03(1)
            for j in range(1, N_YB):
                nc.sync.dma_start(yb[j][:], yT_src[j])
            emit_xT03(2)
            emit_xT03(3)
            emit_xTg(1)
            emit_x16g(0)
            emit_xTg(2)
            emit_x16g(1)
            emit_xTg(3)
            emit_x16g(2)
            emit_x16g(3)
            # warm readback last; its sem-wait never delays real loads.
            nc.sync.dma_start(warm_d, wstage[:])

            # ---- opening: 6 interleaved accumulation chains (m-tiles 0/1
            # x ns 0..2) fed k-step by k-step as each yb tile arrives; the
            # ns=3 groups run as plain groups afterwards on resident data
            # while exp(0,*) frees PSUM banks for the steady state (with 8
            # chains, all stops cluster and G(2,*) stalls on bank WAW). ----
            chain = {(mi, ns): epsp.tile([P, NSLICE], F32, tag="e", name="eps")
                     for mi in (0, 1) for ns in range(NSC)}
            for j in range(N_YB):
                for mi in (0, 1):
                    for kk in range(YKB):
                        k = YKB * j + kk
                        for ns in range(NSC):
                            nc.tensor.matmul(
                                chain[(mi, ns)][:], get_xT(mi, k),
                                yb[j][:, kk, ns * NSLICE:(ns + 1) * NSLICE],
                                start=(k == 0), stop=(k == N_DCHUNK - 1),
                            )
            st0 = emit_open_head(0, [chain[(0, ns)] for ns in range(NSC)])
            ep03 = emit_mm1_group(0, N_NSL - 1)
            st1 = emit_open_head(1, [chain[(1, ns)] for ns in range(NSC)])
            ep13 = emit_mm1_group(1, N_NSL - 1)
            emit_open_tail(0, ep03, *st0)
            emit_open_tail(1, ep13, *st1)

            # ---- steady state ----
            for mi in range(2, N_MTILES):
                eps = [emit_mm1_group(mi, ns) for ns in range(N_NSL)]
                emit_softmax(mi, eps)
            emit_scale(N_MTILES - 2)
            emit_scale(N_MTILES - 1)

            # ---- mm2: accumulate all 16 p.T @ xs contributions per out
            # tile; DVE and ACT stage one 256-col half each in parallel,
            # then a single DMA writes the tile. ----
            HS = NSLICE // 2
            for nch in range(N_NCHUNK):
                for dh in range(N_DHALF):
                    ops = epsp.tile([P, NSLICE], F32, tag="e", name="ops")
                    for mi in range(N_MTILES):
                        rhs = xs[mi]
                        if mi in xsA and nch < NSC * (NSLICE // P):
                            rhs = xsA[mi]
                        nc.tensor.matmul(
                            ops[:],
                            pT[mi][:, nch * P:(nch + 1) * P],
                            rhs[:, dh * NSLICE:(dh + 1) * NSLICE],
                            start=(mi == 0), stop=(mi == N_MTILES - 1),
                        )
                    # separate half tiles: one staging tile would serialize
                    # the DVE and ACT writers (framework WAW ordering)
                    rows = slice(nch * P, (nch + 1) * P)
                    c0 = dh * NSLICE
                    hA = work.tile([P, HS], F32, tag="ostgA", bufs=4)
                    hB = work.tile([P, HS], F32, tag="ostgB", bufs=4)
                    nc.vector.tensor_copy(hA[:], ops[:, 0:HS])
                    nc.scalar.copy(hB[:], ops[:, HS:NSLICE])
                    nc.sync.dma_start(out_d[rows, c0:c0 + HS], hA[:])
                    nc.scalar.dma_start(out_d[rows, c0 + HS:c0 + NSLICE], hB[:])

    nc.compile()
    return nc


_NC_CACHE = {}


def _get_nc():
    if "nc" not in _NC_CACHE:
        _NC_CACHE["nc"] = _build_nc()
    return _NC_CACHE["nc"]


def _host_inputs(x_b: np.ndarray, y_b: np.ndarray) -> dict:
    x16 = x_b.astype(np.float16)
    xTs = np.ascontiguousarray(
        x16.reshape(N_MTILES, P, N_DCHUNK, P).transpose(3, 0, 2, 1))
    return {
        "xTs": xTs,
        "x16": np.ascontiguousarray(x16),
        "yT": np.ascontiguousarray(y_b.astype(np.float16).T),
    }


def kernel(x: np.ndarray, y: np.ndarray) -> np.ndarray:
    assert x.shape == (B, M, D) and y.shape == (B, N, D)
    nc = _get_nc()
    in_maps = [_host_inputs(x[b], y[b]) for b in range(B)]
    res = bass_utils.run_bass_kernel_spmd(nc, in_maps, core_ids=list(range(B)))
    return np.stack([res.results[b]["out"] for b in range(B)], axis=0)
